# revision 1
# baseline (speedup 1.0000x reference)
"""ContentGuidedAttention Trainium2 kernel.

Full NxN single-head cross-attention + out-proj + residual + LayerNorm,
for B=4, C=256, H=W=64 (N=4096 tokens), distributed over 8 NeuronCores:
core i handles batch i//2, query-half i%2 (2048 queries, all 4096 keys).
No collectives: K/V are computed redundantly on the two cores sharing a
batch (~5% extra FLOPs).

Layout strategy (all channel-major, zero transposes, fp8 everywhere the
PE touches data):
  - high/low(copy)/weights are cast to fp8e4 host-side; q/k weights are
    prescaled by 16 so fp8 sees well-ranged values (compensated in the
    softmax exp scale). All projections run as fp8 DoubleRow matmuls
    (contract 256 in one PE pass).
  - Q^T, K^T computed as [C, n] (channels on partitions) fp8e4
  - V computed token-major [n, C] fp8e4
  - S^T = K Q^T via fp8 DoubleRow; exp on ACT -> P^T in fp8e4
  - softmax denominator: DoubleRow ones-vector matmuls accumulate
    sum_k P^T[k, q] into a [1, q] psum row (no DVE tree at all)
  - reciprocals and rsqrt run on ACT as exp(-ln x) / exp(-0.5 ln x):
    Ln and Exp share one activation-table set, so no table switches
  - row -> all-partition replication via gpsimd partition_broadcast
  - PV: O^T[c, q] = sum_k V[k,c] P^T[k,q], fp8 DoubleRow over key pairs;
    O^T (max |val| ~134 < fp8 max 240) is evacuated to fp8 so the
    out-proj is DoubleRow too
  - residual uses a separate f32 copy of low; LN entirely per-query-
    block, overlapped with the next block's attention
"""

import ml_dtypes
import numpy as np

import concourse.bass as bass
import concourse.mybir as mybir
import concourse.tile as tile
from concourse import bacc
from concourse.bass import ds, ts
from concourse.bass_utils import run_bass_kernel_spmd

F32 = mybir.dt.float32
F32R = mybir.dt.float32r
BF16 = mybir.dt.bfloat16
F8 = mybir.dt.float8e4
AF = mybir.ActivationFunctionType
OP = mybir.AluOpType
DR = mybir.MatmulPerfMode.DoubleRow

B = 4
C = 256
N = 4096          # tokens per batch
NQ = 2048         # queries per core
QB = 512          # query block
NQB = NQ // QB    # 4
NKC = N // 128    # 32 key chunks
NKR = 4           # key ranges (1024 keys each) for K^T / V tiles
QK_PRE = 16.0     # host-side prescale on q/k weights (fp8 range centering)
SCALE = (C // 8) ** -0.5
EXP_SCALE = SCALE / (QK_PRE * QK_PRE)
LN_EPS = 1e-5

_CACHE = {}


def _build_nc():
    nc = bacc.Bacc("TRN2", target_bir_lowering=False, debug=False)

    low_d = nc.declare_dram_parameter("low", [C, NQ], F32R, isOutput=False)
    lowf8_d = nc.declare_dram_parameter("lowf8", [C, NQ], F8, isOutput=False)
    high_d = nc.declare_dram_parameter("high", [C, N], F8, isOutput=False)
    # weights are passed pre-transposed: [c_in, c_out], fp8
    wq_d = nc.declare_dram_parameter("wq", [C, C], F8, isOutput=False)
    wk_d = nc.declare_dram_parameter("wk", [C, C], F8, isOutput=False)
    wv_d = nc.declare_dram_parameter("wv", [C, C], F8, isOutput=False)
    wo_d = nc.declare_dram_parameter("wo", [C, C], F8, isOutput=False)
    # qb, kb, ob, lng, lnb prepacked host-side as [128, 10]
    pvec_d = nc.declare_dram_parameter("pvec", [128, 10], F32, isOutput=False)
    out_d = nc.declare_dram_parameter("out", [C, NQ], F32, isOutput=True)

    with tile.TileContext(nc) as tc:
        with (
            tc.tile_pool(name="persist", bufs=1) as pp,
            tc.tile_pool(name="high", bufs=4) as high_pool,
            tc.tile_pool(name="pt", bufs=8) as pt_pool,
            tc.tile_pool(name="ot", bufs=3) as ot_pool,
            tc.tile_pool(name="scratch", bufs=3) as scr_pool,
            tc.tile_pool(name="rowscr", bufs=1) as row_pool,
            tc.tile_pool(name="outsb", bufs=4) as out_pool,
            tc.tile_pool(name="st_ps", bufs=2, space="PSUM") as st_ps,
            tc.tile_pool(name="acc_ps", bufs=3, space="PSUM") as acc_ps,
            tc.tile_pool(name="row_ps", bufs=1, space="PSUM") as row_ps,
        ):
            # ---------------- constants / parameters ----------------
            # one tile per weight matrix ([cin_p, cin_chunk, cout]); K/V
            # weights load first so the K/V projections start ASAP
            wk_sb = pp.tile([128, 2, C], F8)
            wv_sb = pp.tile([128, 2, C], F8)
            wq_sb = pp.tile([128, 2, C], F8)
            wo_sb = pp.tile([128, 2, C], F8)
            for j in range(2):
                nc.scalar.dma_start(out=wk_sb[:, j, :], in_=wk_d[ds(j * 128, 128), :])
            pvec = pp.tile([128, 10], F32)
            nc.scalar.dma_start(out=pvec[:, :], in_=pvec_d[:, :])
            for j in range(2):
                nc.scalar.dma_start(out=wq_sb[:, j, :], in_=wq_d[ds(j * 128, 128), :])
            lowf8_sb = pp.tile([128, 2, NQ], F8)

            # memset cannot emit float32r/fp8; stage in f32 and copy
            stage = pp.tile([128, 128], F32)
            ones128 = pp.tile([128, 1], F32R)    # partition-reduce lhsT (f32r)
            nc.vector.memset(stage[:, 0:1], 1.0)
            nc.vector.tensor_copy(ones128[:, :], stage[:, 0:1])
            ones2f8 = pp.tile([128, 2, 16], F8)  # DoubleRow denom lhsT
            nc.vector.memset(stage[:, 0:2], 1.0)
            nc.vector.tensor_copy(ones2f8[:, :, 0], stage[:, 0:2])
            epsb = pp.tile([1, 1], F32)          # LN epsilon bias
            nc.vector.memset(epsb[:, :], LN_EPS)
            ones_col = pp.tile([1, 128], F32R)   # K=1 row-broadcast lhsT
            nc.vector.memset(stage[0:1, :], 1.0)
            nc.vector.tensor_copy(ones_col[:, :], stage[0:1, :])


            QBIAS, KBIAS, OBIAS, LNG, LNB = 0, 2, 4, 6, 8

            # ---------------- K^T / V projections ----------------
            # per 1024-key-range tiles so attention can start early
            kt_sb = [
                pp.tile([128, 2, 1024], F8, name=f"kt{r}", tag=f"kt{r}")
                for r in range(NKR)
            ]
            v_sb = [
                pp.tile([128, 8, C], F8, name=f"v{r}", tag=f"v{r}")
                for r in range(NKR)
            ]
            hi_tiles = []
            for r in range(NKR):
                hi = high_pool.tile([128, 2, 1024], F8)
                for j in range(2):
                    nc.sync.dma_start(
                        out=hi[:, j, :],
                        in_=high_d[ds(j * 128, 128), ds(r * 1024, 1024)],
                    )
                hi_tiles.append(hi)
                if r == 0:
                    # lowf8 + wv ride early on the sync ring: lowf8 lets
                    # block 0's Q projection (and so its S matmuls + exps)
                    # start ~12us in; wv lets V matmuls fill the exp-paced
                    # PE gaps during block 0's S phase
                    for j in range(2):
                        nc.sync.dma_start(
                            out=lowf8_sb[:, j, :],
                            in_=lowf8_d[ds(j * 128, 128), :],
                        )
                    for j in range(2):
                        nc.sync.dma_start(
                            out=wv_sb[:, j, :], in_=wv_d[ds(j * 128, 128), :]
                        )
            # wv/wo/low ride the sync ring behind high: none is needed
            # before ~20us, and this keeps the scalar ring short so
            # wq/lowf8 land early and block 0's S matmuls start ~12us.
            for j in range(2):
                nc.sync.dma_start(out=wo_sb[:, j, :], in_=wo_d[ds(j * 128, 128), :])
            low_sb = pp.tile([128, 2, NQ], F32R)
            for j in range(2):
                nc.sync.dma_start(
                    out=low_sb[:, j, :], in_=low_d[ds(j * 128, 128), :]
                )

            def k_range(r):
                hi = hi_tiles[r]
                # K^T: out [cout, k] = sum_cin wk[cin, cout] high[cin, k]
                for h in range(2):
                    for c in range(2):
                        kps = st_ps.tile([128, 512], F32, tag="st")
                        if r == 0 and h == 0:
                            # per-j split: the very first matmul only needs
                            # the first wk/hi DMA chunks -> earlier PE start
                            for j in range(2):
                                nc.tensor.matmul(
                                    out=kps[:, :],
                                    lhsT=wk_sb[:, j, ds(c * 128, 128)],
                                    rhs=hi[:, j, ds(h * 512, 512)],
                                    start=(j == 0), stop=(j == 1),
                                )
                        else:
                            nc.tensor.matmul(
                                out=kps[:, :],
                                lhsT=wk_sb[:, :, ds(c * 128, 128)],
                                rhs=hi[:, :, ds(h * 512, 512)],
                                start=True, stop=True,
                                perf_mode=DR,
                            )
                        nc.vector.tensor_scalar_add(
                            out=kt_sb[r][:, c, ds(h * 512, 512)],
                            in0=kps[:, :],
                            scalar1=pvec[:, ds(KBIAS + c, 1)],
                        )

            def v_range(r):
                # V: out [k, cout] = sum_cin high[cin, k] wv[cin, cout]
                # plain fp8 (not DoubleRow): FWL weight loads beat
                # DoubleRow's 2x-long LDWEIGHTS at this free dim
                hi = hi_tiles[r]
                for u in range(8):
                    vps = st_ps.tile([128, C], F32, tag="st")
                    for j in range(2):
                        nc.tensor.matmul(
                            out=vps[:, :],
                            lhsT=hi[:, j, ds(u * 128, 128)],
                            rhs=wv_sb[:, j, :],
                            start=(j == 0), stop=(j == 1),
                        )
                    nc.vector.tensor_copy(v_sb[r][:, u, :], vps[:, :])

            qt_all = pp.tile([128, 2, NQ], F8)

            def q_proj(qb4):
                for c in range(2):
                    qps = st_ps.tile([128, QB], F32, tag="st")
                    nc.tensor.matmul(
                        out=qps[:, :],
                        lhsT=wq_sb[:, :, ds(c * 128, 128)],
                        rhs=lowf8_sb[:, :, ds(qb4 * QB, QB)],
                        start=True, stop=True,
                        perf_mode=DR,
                    )
                    nc.vector.tensor_scalar_add(
                        out=qt_all[:, c, ds(qb4 * QB, QB)], in0=qps[:, :],
                        scalar1=pvec[:, ds(QBIAS + c, 1)],
                    )

            # ---------------- main loop over query blocks ----------------
            # PE order per block: 32 DoubleRow S matmuls, 16 DoubleRow
            # denominator matmuls (into a psum row, so the recip chain
            # starts while PV still runs), 32 DoubleRow PV matmuls.

            def alloc_quarters(b):
                return [
                    pt_pool.tile([128, 8, QB], F8, tag="ptq", name=f"ptq{g}")
                    for g in range(4)
                ]

            def attn_group(b, quarters, si0, si1):
                qsl = ds(b * QB, QB)
                for si in range(si0, si1):
                    sps = st_ps.tile([128, 2, QB], F32, tag="st")
                    for u in range(2):
                        kc = si * 2 + u
                        nc.tensor.matmul(
                            out=sps[:, u, :],
                            lhsT=kt_sb[kc // 8][:, :, ds((kc % 8) * 128, 128)],
                            rhs=qt_all[:, :, qsl],
                            start=True, stop=True,
                            perf_mode=DR,
                        )
                    nc.scalar.activation(
                        out=quarters[si // 4][:, ds((si % 4) * 2, 2), :],
                        in_=sps[:, :, :],
                        func=AF.Exp,
                        scale=EXP_SCALE,
                    )

            def attention(b):
                quarters = alloc_quarters(b)
                attn_group(b, quarters, 0, 16)
                return quarters

            def denom(b, quarters):
                dps = row_ps.tile([1, QB], F32, tag="row")
                for t in range(16):
                    nc.tensor.matmul(
                        out=dps[:, :],
                        lhsT=ones2f8[:, :, 0:1],
                        rhs=quarters[t // 4][:, ds((t % 4) * 2, 2), :],
                        start=(t == 0), stop=(t == 15),
                        perf_mode=DR,
                        skip_group_check=True,
                    )
                return dps

            def pv(b, quarters):
                ot = ot_pool.tile([128, 2, QB], F8, tag="ot", name=f"ot{b}")
                for c in range(2):
                    ops = acc_ps.tile([128, QB], F32, tag="acc")
                    for t in range(16):
                        nc.tensor.matmul(
                            out=ops[:, :],
                            lhsT=v_sb[t // 4][:, ds((t % 4) * 2, 2),
                                             ds(c * 128, 128)],
                            rhs=quarters[t // 4][:, ds((t % 4) * 2, 2), :],
                            start=(t == 0), stop=(t == 15),
                            perf_mode=DR,
                            skip_group_check=True,
                        )
                    nc.vector.tensor_copy(ot[:, c, :], ops[:, :])
                return ot

            def denom_recip(b, dps):
                # 1/denom = exp(-ln(denom)) on ACT (same table set as Exp)
                lnrow = row_pool.tile([1, QB], F32, tag="lnrow")
                nc.scalar.activation(
                    out=lnrow[:, :], in_=dps[:, :], func=AF.Ln
                )
                rcprow = row_pool.tile([1, QB], F32, tag="rcprow",
                                       name=f"rcprow{b}")
                nc.scalar.activation(
                    out=rcprow[:, :], in_=lnrow[:, :], func=AF.Exp, scale=-1.0
                )
                rcp_rep = scr_pool.tile([128, QB], F32, tag="rcprep",
                                        name=f"rcprep{b}")
                nc.gpsimd.partition_broadcast(rcp_rep[:, :], rcprow[:, :])
                return rcprow, rcp_rep

            def outproj_y(b, ot, rcp_rep, qo=0, ql=QB):
                qsl = ds(b * QB + qo, ql)
                y_sb = ot_pool.tile([128, 2, ql], F32R, tag="y",
                                    name=f"y{b}_{qo}")
                for c in range(2):
                    pps = acc_ps.tile([128, ql], F32, tag="acc")
                    nc.tensor.matmul(
                        out=pps[:, :],
                        lhsT=wo_sb[:, :, ds(c * 128, 128)],
                        rhs=ot[:, :, ds(qo, ql)],
                        start=True, stop=True,
                        perf_mode=DR,
                    )
                    ysc = scr_pool.tile([128, ql], F32, tag="scr")
                    nc.vector.tensor_mul(
                        out=ysc[:, :], in0=pps[:, :], in1=rcp_rep[:, ds(qo, ql)]
                    )
                    nc.vector.scalar_tensor_tensor(
                        out=y_sb[:, c, :],
                        in0=ysc[:, :],
                        scalar=pvec[:, ds(OBIAS + c, 1)],
                        in1=low_sb[:, c, qsl].bitcast(F32),
                        op0=OP.add, op1=OP.add,
                    )
                return y_sb

            def stats_ln(b, y_sb, qo=0, ql=QB, last=False):
                qsl = ds(b * QB + qo, ql)
                sy_ps = row_ps.tile([1, ql], F32, tag="row")
                for c in range(2):
                    nc.tensor.matmul(
                        out=sy_ps[:, :],
                        lhsT=ones128[:, :],
                        rhs=y_sb[:, c, :],
                        start=(c == 0), stop=(c == 1),
                    )
                murow = row_pool.tile([1, ql], F32, tag="murow")
                if last:
                    nc.scalar.activation(
                        out=murow[:, :], in_=sy_ps[:, :], func=AF.Copy,
                        scale=1.0 / C,
                    )
                else:
                    nc.vector.tensor_scalar_mul(
                        out=murow[:, :], in0=sy_ps[:, :], scalar1=1.0 / C
                    )
                sy2_ps = row_ps.tile([1, ql], F32, tag="row")
                for c in range(2):
                    ysq = scr_pool.tile([128, ql], F32R, tag="ysq")
                    nc.vector.tensor_mul(
                        out=ysq[:, :],
                        in0=y_sb[:, c, :].bitcast(F32),
                        in1=y_sb[:, c, :].bitcast(F32),
                    )
                    nc.tensor.matmul(
                        out=sy2_ps[:, :],
                        lhsT=ones128[:, :],
                        rhs=ysq[:, :],
                        start=(c == 0), stop=(c == 1),
                    )
                # C*var = sy2 - C*mu^2 ; the 1/C rides the Ln scale:
                # rstd = exp(-0.5 ln((C var)/C + eps))
                mu2row = row_pool.tile([1, ql], F32, tag="mu2row")
                nc.vector.tensor_mul(
                    out=mu2row[:, :], in0=murow[:, :], in1=murow[:, :],
                )
                varrow = row_pool.tile([1, ql], F32, tag="varrow")
                nc.vector.scalar_tensor_tensor(
                    out=varrow[:, :], in0=mu2row[:, :], scalar=-float(C),
                    in1=sy2_ps[:, :], op0=OP.mult, op1=OP.add,
                )
                lnv = row_pool.tile([1, ql], F32, tag="lnv")
                nc.scalar.activation(
                    out=lnv[:, :], in_=varrow[:, :], func=AF.Ln,
                    scale=1.0 / C, bias=epsb[:, :],
                )
                rstdrow = row_pool.tile([1, ql], F32R if last else F32,
                                        tag="rstdrow")
                nc.scalar.activation(
                    out=rstdrow[:, :], in_=lnv[:, :], func=AF.Exp, scale=-0.5
                )
                mu_rep = scr_pool.tile([128, ql], F32, tag="murep")
                nc.gpsimd.partition_broadcast(mu_rep[:, :], murow[:, :])
                if last:
                    # span-critical tail: broadcast rstd via a K=1 PE matmul
                    # into psum (~0.3us vs ~1us gpsimd) and do the final
                    # affine on the otherwise-idle ACT so DVE stops being
                    # the serial bottleneck of the last-block chain
                    rs_ps = acc_ps.tile([128, ql], F32, tag="acc")
                    nc.tensor.matmul(
                        out=rs_ps[:, :], lhsT=ones_col[:, :],
                        rhs=rstdrow[:, :], start=True, stop=True,
                    )
                    rs_rep = rs_ps
                else:
                    rs_rep = scr_pool.tile([128, ql], F32, tag="rsrep")
                    nc.gpsimd.partition_broadcast(rs_rep[:, :], rstdrow[:, :])
                for c in range(2):
                    yn = scr_pool.tile([128, ql], F32, tag="scr")
                    nc.vector.tensor_sub(
                        out=yn[:, :],
                        in0=y_sb[:, c, :].bitcast(F32),
                        in1=mu_rep[:, :],
                    )
                    nc.vector.tensor_mul(
                        out=yn[:, :], in0=yn[:, :], in1=rs_rep[:, :]
                    )
                    osb = out_pool.tile([128, ql], F32)
                    nc.vector.tensor_scalar(
                        out=osb[:, :], in0=yn[:, :],
                        scalar1=pvec[:, ds(LNG + c, 1)],
                        scalar2=pvec[:, ds(LNB + c, 1)],
                        op0=OP.mult, op1=OP.add,
                    )
                    nc.sync.dma_start(
                        out=out_d[ds(c * 128, 128), qsl], in_=osb[:, :]
                    )

            # --- block 0, software-pipelined with the projection preamble:
            # K ranges interleave with S groups so exps start ~12us in;
            # V projections and the remaining Q blocks fill the PE slack
            # while ACT streams block 0's exps.
            k_range(0)
            q_proj(0)
            q0 = alloc_quarters(0)
            attn_group(0, q0, 0, 4)
            k_range(1)
            attn_group(0, q0, 4, 8)
            k_range(2)
            attn_group(0, q0, 8, 12)
            k_range(3)
            attn_group(0, q0, 12, 16)
            for r in range(NKR):
                v_range(r)
            for qb4 in range(1, NQB):
                q_proj(qb4)
            quarters = {0: q0}
            dps = {0: denom(0, q0)}

            # --- steady state: weave block nb's S matmuls (which feed the
            # ACT exp stream) into block b's PV/out-proj/LN emission, so
            # the PE queue never serializes a whole PV phase in front of
            # the next block's exps. denom_recip(b) is emitted first so
            # its Ln/Exp rows precede block nb's exps in the ACT queue.
            for b in range(NQB):
                nb = b + 1
                rcprow, rcp_rep = denom_recip(b, dps[b])
                if nb < NQB:
                    quarters[nb] = alloc_quarters(nb)
                ot = ot_pool.tile([128, 2, QB], F8, tag="ot", name=f"ot{b}")
                for c in range(2):
                    ops = acc_ps.tile([128, QB], F32, tag="acc")
                    for t in range(16):
                        nc.tensor.matmul(
                            out=ops[:, :],
                            lhsT=v_sb[t // 4][:, ds((t % 4) * 2, 2),
                                             ds(c * 128, 128)],
                            rhs=quarters[b][t // 4][:, ds((t % 4) * 2, 2), :],
                            start=(t == 0), stop=(t == 15),
                            perf_mode=DR,
                            skip_group_check=True,
                        )
                    if nb == NQB:
                        # tail: evacuate on the idle ACT so DVE's serial
                        # out-proj/LN chain starts sooner
                        nc.scalar.activation(
                            out=ot[:, c, :], in_=ops[:, :], func=AF.Copy
                        )
                    else:
                        nc.vector.tensor_copy(ot[:, c, :], ops[:, :])
                    if nb < NQB:
                        attn_group(nb, quarters[nb], c * 4, c * 4 + 4)
                y_b = outproj_y(b, ot, rcp_rep)
                if nb < NQB:
                    attn_group(nb, quarters[nb], 8, 12)
                stats_ln(b, y_b, last=(nb == NQB))
                if nb < NQB:
                    attn_group(nb, quarters[nb], 12, 16)
                    dps[nb] = denom(nb, quarters[nb])

    # Force Exp and Ln to resolve to the one table set containing both
    # (the default chooser alternates exp_and_others <-> natural_log_exp,
    # paying a ~1.3us table load per switch, ~17 loads per kernel).
    import bass_rust as _br
    from concourse.hw_specs import get_activation_tables as _gat

    def _patched_act_loads():
        has_act = any(
            isinstance(i, mybir.InstActivation)
            for blk in nc.main_func.blocks for i in blk.instructions
        )
        if not has_act:
            return
        tables = []
        for name, fns in _gat(nc.m.arch).items():
            if name != "natural_log_exp_and_others":
                fns = fns - {AF.Exp, AF.Ln}
            tables.append((name, fns))
        _br.insert_act_table_loads(nc, tables)

    nc.insert_act_table_loads = _patched_act_loads
    nc.compile()
    return nc


def get_nc():
    if "nc" not in _CACHE:
        _CACHE["nc"] = _build_nc()
    return _CACHE["nc"]


def make_in_maps(low, high, q_w, q_b, k_w, k_b, v_w, v_b, o_w, o_b, ln_g, ln_b):
    low_r = np.asarray(low, np.float32).reshape(B, C, N)
    high_r = np.asarray(high, np.float32).reshape(B, C, N)
    f32 = lambda x: np.ascontiguousarray(np.asarray(x, np.float32))
    f8 = lambda x: np.ascontiguousarray(
        np.asarray(x, np.float32).astype(ml_dtypes.float8_e4m3)
    )
    # v-bias is exactly equivalent to an out-proj bias shift because the
    # softmax rows sum to one: attn @ (V + 1 vb^T) @ o_w^T = attn @ V @ o_w^T
    # + (o_w @ v_b)^T, so fold it on the host.
    ob_eff = np.asarray(o_b, np.float32) + np.asarray(o_w, np.float32) @ np.asarray(v_b, np.float32)
    pv_cols = []
    for v in [np.asarray(q_b, np.float32) * QK_PRE,
              np.asarray(k_b, np.float32) * QK_PRE,
              ob_eff, ln_g, ln_b]:
        pv_cols.append(np.asarray(v, np.float32).reshape(2, 128).T)
    shared = {
        "wq": f8(np.asarray(q_w, np.float32).T * QK_PRE),
        "wk": f8(np.asarray(k_w, np.float32).T * QK_PRE),
        "wv": f8(np.asarray(v_w, np.float32).T),
        "wo": f8(np.asarray(o_w, np.float32).T),
        "pvec": f32(np.concatenate(pv_cols, axis=1)),
    }
    in_maps = []
    for i in range(8):
        bidx, h = i // 2, i % 2
        lo = low_r[bidx][:, h * NQ:(h + 1) * NQ]
        in_maps.append({
            "low": f32(lo),
            "lowf8": f8(lo),
            "high": f8(high_r[bidx]),
            **shared,
        })
    return in_maps


def assemble(results):
    out = np.empty((B, C, N), np.float32)
    for i in range(8):
        bidx, h = i // 2, i % 2
        out[bidx][:, h * NQ:(h + 1) * NQ] = results[i]["out"]
    return out.reshape(B, C, 64, 64)


def kernel(**inputs) -> np.ndarray:
    nc = get_nc()
    in_maps = make_in_maps(**inputs)
    res = run_bass_kernel_spmd(nc, in_maps, core_ids=list(range(8)))
    return assemble(res.results)


if __name__ == "__main__":
    pass



# revision 5
# speedup vs baseline: 1.0283x; 1.0283x over previous
"""ContentGuidedAttention Trainium2 kernel.

Full NxN single-head cross-attention + out-proj + residual + LayerNorm,
for B=4, C=256, H=W=64 (N=4096 tokens), distributed over 8 NeuronCores:
core i handles batch i//2, query-half i%2 (2048 queries, all 4096 keys).
No collectives: K/V are computed redundantly on the two cores sharing a
batch (~5% extra FLOPs).

Layout strategy (channel-major, zero transposes, fp8 DoubleRow on every
matmul the PE streams):
  - Q^T/K^T as [C, n] fp8e4 (q/k weights prescaled by 16, compensated in
    the softmax exp scale); V token-major [n, C] fp8e4 via DR matmuls,
    evacuated on GpSimd (DVE is the preamble bottleneck)
  - S^T = K Q^T fp8 DR; exp on ACT -> P^T fp8e4
  - softmax denominator: DR ones-vector matmuls accumulate into a [1, q]
    psum row; 1/denom via a single DVE reciprocal_approx_fast (keeps the
    ACT queue exp-only), then gpsimd partition_broadcast
  - PV: O^T[c, q] = sum_k V[k,c] P^T[k,q], fp8 DR; O^T evacuated to fp8
    so the out-proj is DR too; residual uses a separate f32 copy of low
  - scheduling: the steady-state loop WEAVES 2-si S-matmul pairs between
    ~1.5us chunks of PV/outproj/LN/denominator matmuls so the in-order
    PE queue never parks behind an S matmul whose psum buffer is still
    being drained by ACT (st_ps has only 2 bufs), and ACT always has exp
    work queued.  Same fine-grained interleave in the projection
    preamble.  ~35 dummy 128-col matmuls at t~3.5us warm the PE HAM
    clock-gate before real work; inputs ride 4 DMA queues in parallel.
  - tail: last block's out-proj + LN run as two 256-query halves so the
    serial ACT/DVE chain of one half overlaps the other's matmuls.
"""

import ml_dtypes
import numpy as np

import concourse.bass as bass
import concourse.mybir as mybir
import concourse.tile as tile
from concourse import bacc
from concourse.bass import ds, ts
from concourse.bass_utils import run_bass_kernel_spmd

F32 = mybir.dt.float32
F32R = mybir.dt.float32r
BF16 = mybir.dt.bfloat16
F8 = mybir.dt.float8e4
AF = mybir.ActivationFunctionType
OP = mybir.AluOpType
DR = mybir.MatmulPerfMode.DoubleRow

B = 4
C = 256
N = 4096          # tokens per batch
NQ = 2048         # queries per core
QB = 512          # query block
NQB = NQ // QB    # 4
NKC = N // 128    # 32 key chunks
NKR = 4           # key ranges (1024 keys each) for K^T / V tiles
QK_PRE = 16.0     # host-side prescale on q/k weights (fp8 range centering)
SCALE = (C // 8) ** -0.5
EXP_SCALE = SCALE / (QK_PRE * QK_PRE)
LN_EPS = 1e-5

_CACHE = {}


def _build_nc():
    nc = bacc.Bacc("TRN2", target_bir_lowering=False, debug=False)

    low_d = nc.declare_dram_parameter("low", [C, NQ], F32R, isOutput=False)
    lowf8_d = nc.declare_dram_parameter("lowf8", [C, NQ], F8, isOutput=False)
    high_d = nc.declare_dram_parameter("high", [C, N], F8, isOutput=False)
    # weights are passed pre-transposed: [c_in, c_out], fp8
    wq_d = nc.declare_dram_parameter("wq", [C, C], F8, isOutput=False)
    wk_d = nc.declare_dram_parameter("wk", [C, C], F8, isOutput=False)
    wv_d = nc.declare_dram_parameter("wv", [C, C], F8, isOutput=False)
    wo_d = nc.declare_dram_parameter("wo", [C, C], F8, isOutput=False)
    # qb, kb, ob, lng, lnb prepacked host-side as [128, 10]
    pvec_d = nc.declare_dram_parameter("pvec", [128, 10], F32, isOutput=False)
    out_d = nc.declare_dram_parameter("out", [C, NQ], F32, isOutput=True)

    with tile.TileContext(nc) as tc:
        with (
            tc.tile_pool(name="persist", bufs=1) as pp,
            tc.tile_pool(name="high", bufs=4) as high_pool,
            tc.tile_pool(name="pt", bufs=8) as pt_pool,
            tc.tile_pool(name="ot", bufs=3) as ot_pool,
            tc.tile_pool(name="scratch", bufs=3) as scr_pool,
            tc.tile_pool(name="rowscr", bufs=1) as row_pool,
            tc.tile_pool(name="outsb", bufs=4) as out_pool,
            tc.tile_pool(name="st_ps", bufs=2, space="PSUM") as st_ps,
            tc.tile_pool(name="acc_ps", bufs=3, space="PSUM") as acc_ps,
            tc.tile_pool(name="row_ps", bufs=1, space="PSUM") as row_ps,
        ):
            # ---------------- constants + PE warm-up ----------------
            stage = pp.tile([128, 128], F32)
            nc.vector.memset(stage[:, :], 1.0)
            ones128 = pp.tile([128, 1], F32R)    # partition-reduce lhsT
            nc.vector.tensor_copy(ones128[:, :], stage[:, 0:1])
            # tiny exp: pulls the ACT table load to ~3.6us (ACT idle)
            tinyrow = pp.tile([1, 1], F32)
            nc.scalar.activation(out=tinyrow[:, :], in_=stage[0:1, 0:1],
                                 func=AF.Exp)
            # ~35 dummy matmuls release the PE HAM clock-gate (~3.4us of
            # sustained activity) before the first real matmul at ~8us
            warm_ps = row_ps.tile([1, 128], F32, tag="row")
            for w in range(35):
                nc.tensor.matmul(
                    out=warm_ps[:, :], lhsT=ones128[:, :],
                    rhs=stage[:, :].bitcast(F32R),
                    start=True, stop=True, skip_group_check=True,
                )
            ones2f8 = pp.tile([128, 2, 16], F8)  # DoubleRow denom lhsT
            nc.vector.tensor_copy(ones2f8[:, :, 0], stage[:, 0:2])
            ones_col = pp.tile([1, 128], F32R)   # K=1 row-broadcast lhsT
            nc.vector.tensor_copy(ones_col[:, :], stage[0:1, :])
            epsb = pp.tile([1, 1], F32)          # LN epsilon bias
            nc.vector.memset(epsb[:, :], LN_EPS)

            # ---------------- input DMAs on 4 parallel queues --------
            wk_sb = pp.tile([128, 2, C], F8)
            wv_sb = pp.tile([128, 2, C], F8)
            wq_sb = pp.tile([128, 2, C], F8)
            wo_sb = pp.tile([128, 2, C], F8)
            pvec = pp.tile([128, 10], F32)
            lowf8_sb = pp.tile([128, 2, NQ], F8)
            low_sb = pp.tile([128, 2, NQ], F32R)
            for j in range(2):
                nc.scalar.dma_start(out=wk_sb[:, j, :], in_=wk_d[ds(j * 128, 128), :])
            nc.gpsimd.dma_start(out=pvec[:, :], in_=pvec_d[:, :])
            for j in range(2):
                nc.gpsimd.dma_start(out=wq_sb[:, j, :], in_=wq_d[ds(j * 128, 128), :])
            for j in range(2):
                nc.gpsimd.dma_start(out=wv_sb[:, j, :], in_=wv_d[ds(j * 128, 128), :])
            for j in range(2):
                nc.gpsimd.dma_start(out=wo_sb[:, j, :], in_=wo_d[ds(j * 128, 128), :])
            hi_tiles = [
                high_pool.tile([128, 2, 1024], F8, name=f"hi{r}")
                for r in range(NKR)
            ]
            # range 0 rides first, split in h-halves so the very first
            # K-projection matmul waits on a [128,512] transfer only
            for h in range(2):
                for j in range(2):
                    nc.sync.dma_start(
                        out=hi_tiles[0][:, j, ds(h * 512, 512)],
                        in_=high_d[ds(j * 128, 128), ds(h * 512, 512)],
                    )
            for j in range(2):
                nc.sync.dma_start(out=lowf8_sb[:, j, :], in_=lowf8_d[ds(j * 128, 128), :])
            for r in range(1, NKR):
                for j in range(2):
                    nc.sync.dma_start(
                        out=hi_tiles[r][:, j, :],
                        in_=high_d[ds(j * 128, 128), ds(r * 1024, 1024)],
                    )
            for j in range(2):
                nc.sync.dma_start(out=low_sb[:, j, :], in_=low_d[ds(j * 128, 128), :])

            QBIAS, KBIAS, OBIAS, LNG, LNB = 0, 2, 4, 6, 8

            kt_sb = [
                pp.tile([128, 2, 1024], F8, name=f"kt{r}", tag=f"kt{r}")
                for r in range(NKR)
            ]
            v_sb = [
                pp.tile([128, 8, C], F8, name=f"v{r}", tag=f"v{r}")
                for r in range(NKR)
            ]
            qt_all = pp.tile([128, 2, NQ], F8)

            # ---------------- work units ----------------
            def k_unit(r, h, split_j=False):
                # K^T: out [cout, k] = sum_cin wk[cin, cout] high[cin, k]
                for c in range(2):
                    kps = st_ps.tile([128, 512], F32, tag="st")
                    if split_j:
                        # first matmuls only need the first DMA chunks
                        for j in range(2):
                            nc.tensor.matmul(
                                out=kps[:, :],
                                lhsT=wk_sb[:, j, ds(c * 128, 128)],
                                rhs=hi_tiles[r][:, j, ds(h * 512, 512)],
                                start=(j == 0), stop=(j == 1),
                            )
                    else:
                        nc.tensor.matmul(
                            out=kps[:, :],
                            lhsT=wk_sb[:, :, ds(c * 128, 128)],
                            rhs=hi_tiles[r][:, :, ds(h * 512, 512)],
                            start=True, stop=True,
                            perf_mode=DR,
                        )
                    # K bias dropped: a k-independent logit shift per query,
                    # exactly cancelled by softmax
                    nc.vector.tensor_copy(
                        kt_sb[r][:, c, ds(h * 512, 512)], kps[:, :]
                    )

            def v_unit(r, up):
                # V: out [k, cout] = sum_cin high[cin, k] wv[cin, cout]
                # DR over the cin halves; last range evacuates on ACT to
                # balance the preamble DVE load
                vps = st_ps.tile([128, 2, C], F32, tag="st")
                for i in range(2):
                    u = up * 2 + i
                    nc.tensor.matmul(
                        out=vps[:, i, :],
                        lhsT=hi_tiles[r][:, :, ds(u * 128, 128)],
                        rhs=wv_sb[:, :, :],
                        start=True, stop=True,
                        perf_mode=DR,
                    )
                if r == NKR - 1:
                    nc.scalar.activation(
                        out=v_sb[r][:, ds(up * 2, 2), :], in_=vps[:, :, :],
                        func=AF.Copy,
                    )
                else:
                    nc.vector.tensor_copy(
                        v_sb[r][:, ds(up * 2, 2), :], vps[:, :, :]
                    )

            def q_proj(qb4):
                for c in range(2):
                    qps = st_ps.tile([128, QB], F32, tag="st")
                    nc.tensor.matmul(
                        out=qps[:, :],
                        lhsT=wq_sb[:, :, ds(c * 128, 128)],
                        rhs=lowf8_sb[:, :, ds(qb4 * QB, QB)],
                        start=True, stop=True,
                        perf_mode=DR,
                    )
                    nc.vector.tensor_scalar_add(
                        out=qt_all[:, c, ds(qb4 * QB, QB)], in0=qps[:, :],
                        scalar1=pvec[:, ds(QBIAS + c, 1)],
                    )

            def alloc_quarters(b):
                return [
                    pt_pool.tile([128, 8, QB], F8, tag="ptq", name=f"ptq{g}")
                    for g in range(4)
                ]

            quarters = {}

            def s_pair(b, p):
                # 2 si = 4 key chunks: 4 S matmuls + 2 exps; sized so the
                # two st_ps bufs never park the in-order PE queue
                qsl = ds(b * QB, QB)
                for si in (2 * p, 2 * p + 1):
                    sps = st_ps.tile([128, 2, QB], F32, tag="st")
                    for u in range(2):
                        kc = si * 2 + u
                        nc.tensor.matmul(
                            out=sps[:, u, :],
                            lhsT=kt_sb[kc // 8][:, :, ds((kc % 8) * 128, 128)],
                            rhs=qt_all[:, :, qsl],
                            start=True, stop=True,
                            perf_mode=DR,
                        )
                    nc.scalar.activation(
                        out=quarters[b][si // 4][:, ds((si % 4) * 2, 2), :],
                        in_=sps[:, :, :],
                        func=AF.Exp,
                        scale=EXP_SCALE,
                    )

            def denom(b):
                dps = row_ps.tile([1, QB], F32, tag="row")
                for t in range(16):
                    nc.tensor.matmul(
                        out=dps[:, :],
                        lhsT=ones2f8[:, :, 0:1],
                        rhs=quarters[b][t // 4][:, ds((t % 4) * 2, 2), :],
                        start=(t == 0), stop=(t == 15),
                        perf_mode=DR,
                        skip_group_check=True,
                    )
                return dps

            def pv_part(b, c, t0, t1, ops):
                for t in range(t0, t1):
                    nc.tensor.matmul(
                        out=ops[:, :],
                        lhsT=v_sb[t // 4][:, ds((t % 4) * 2, 2),
                                         ds(c * 128, 128)],
                        rhs=quarters[b][t // 4][:, ds((t % 4) * 2, 2), :],
                        start=(t == 0), stop=(t == t1 - 1),
                        perf_mode=DR,
                        skip_group_check=True,
                    )

            def outproj_y(b, ot, rcp_rep, qo=0, ql=QB):
                qsl = ds(b * QB + qo, ql)
                y_sb = ot_pool.tile([128, 2, ql], F32R, tag="y",
                                    name=f"y{b}_{qo}")
                for c in range(2):
                    pps = acc_ps.tile([128, ql], F32, tag="acc")
                    nc.tensor.matmul(
                        out=pps[:, :],
                        lhsT=wo_sb[:, :, ds(c * 128, 128)],
                        rhs=ot[:, :, ds(qo, ql)],
                        start=True, stop=True,
                        perf_mode=DR,
                    )
                    ysc = scr_pool.tile([128, ql], F32, tag="scr")
                    nc.vector.tensor_mul(
                        out=ysc[:, :], in0=pps[:, :], in1=rcp_rep[:, ds(qo, ql)]
                    )
                    nc.vector.scalar_tensor_tensor(
                        out=y_sb[:, c, :],
                        in0=ysc[:, :],
                        scalar=pvec[:, ds(OBIAS + c, 1)],
                        in1=low_sb[:, c, qsl].bitcast(F32),
                        op0=OP.add, op1=OP.add,
                    )
                return y_sb

            def stats_ln_a(b, y_sb):
                sy_ps = row_ps.tile([1, QB], F32, tag="row")
                for c in range(2):
                    nc.tensor.matmul(
                        out=sy_ps[:, :], lhsT=ones128[:, :],
                        rhs=y_sb[:, c, :], start=(c == 0), stop=(c == 1),
                    )
                murow = row_pool.tile([1, QB], F32, tag="murow")
                nc.vector.tensor_scalar_mul(
                    out=murow[:, :], in0=sy_ps[:, :], scalar1=1.0 / C
                )
                mu_rep = scr_pool.tile([128, QB], F32, tag="murep")
                nc.gpsimd.partition_broadcast(mu_rep[:, :], murow[:, :])
                return murow, mu_rep

            def stats_ln_b(b, y_sb, murow):
                sy2_ps = row_ps.tile([1, QB], F32, tag="row")
                for c in range(2):
                    ysq = scr_pool.tile([128, QB], F32R, tag="ysq")
                    nc.vector.tensor_mul(
                        out=ysq[:, :],
                        in0=y_sb[:, c, :].bitcast(F32),
                        in1=y_sb[:, c, :].bitcast(F32),
                    )
                    nc.tensor.matmul(
                        out=sy2_ps[:, :], lhsT=ones128[:, :],
                        rhs=ysq[:, :], start=(c == 0), stop=(c == 1),
                    )
                # C*var = sy2 - C*mu^2 ; rstd = exp(-0.5 ln((C var)/C + eps))
                mu2row = row_pool.tile([1, QB], F32, tag="mu2row")
                nc.vector.tensor_mul(
                    out=mu2row[:, :], in0=murow[:, :], in1=murow[:, :],
                )
                varrow = row_pool.tile([1, QB], F32, tag="varrow")
                nc.vector.scalar_tensor_tensor(
                    out=varrow[:, :], in0=mu2row[:, :], scalar=-float(C),
                    in1=sy2_ps[:, :], op0=OP.mult, op1=OP.add,
                )
                lnv = row_pool.tile([1, QB], F32, tag="lnv")
                nc.scalar.activation(
                    out=lnv[:, :], in_=varrow[:, :], func=AF.Ln,
                    scale=1.0 / C, bias=epsb[:, :],
                )
                rstdrow = row_pool.tile([1, QB], F32, tag="rstdrow")
                nc.scalar.activation(
                    out=rstdrow[:, :], in_=lnv[:, :], func=AF.Exp, scale=-0.5
                )
                rs_rep = scr_pool.tile([128, QB], F32, tag="rsrep")
                nc.gpsimd.partition_broadcast(rs_rep[:, :], rstdrow[:, :])
                return rs_rep

            def stats_ln_c(b, y_sb, mu_rep, rs_rep):
                qsl = ds(b * QB, QB)
                for c in range(2):
                    yn = scr_pool.tile([128, QB], F32, tag="scr")
                    nc.vector.tensor_sub(
                        out=yn[:, :],
                        in0=y_sb[:, c, :].bitcast(F32),
                        in1=mu_rep[:, :],
                    )
                    nc.vector.tensor_mul(
                        out=yn[:, :], in0=yn[:, :], in1=rs_rep[:, :]
                    )
                    osb = out_pool.tile([128, QB], F32)
                    nc.vector.tensor_scalar(
                        out=osb[:, :], in0=yn[:, :],
                        scalar1=pvec[:, ds(LNG + c, 1)],
                        scalar2=pvec[:, ds(LNB + c, 1)],
                        op0=OP.mult, op1=OP.add,
                    )
                    nc.sync.dma_start(
                        out=out_d[ds(c * 128, 128), qsl], in_=osb[:, :]
                    )

            def stats_ln_last(b, y_sb, qo, ql):
                # span-critical tail: murow on ACT, rstd broadcast via a
                # K=1 PE matmul into psum (~0.3us vs ~1us gpsimd)
                qsl = ds(b * QB + qo, ql)
                sy_ps = row_ps.tile([1, ql], F32, tag="row")
                for c in range(2):
                    nc.tensor.matmul(
                        out=sy_ps[:, :], lhsT=ones128[:, :],
                        rhs=y_sb[:, c, :], start=(c == 0), stop=(c == 1),
                    )
                murow = row_pool.tile([1, ql], F32, tag="murow")
                nc.scalar.activation(
                    out=murow[:, :], in_=sy_ps[:, :], func=AF.Copy,
                    scale=1.0 / C,
                )
                sy2_ps = row_ps.tile([1, ql], F32, tag="row")
                for c in range(2):
                    ysq = scr_pool.tile([128, ql], F32R, tag="ysq")
                    nc.vector.tensor_mul(
                        out=ysq[:, :],
                        in0=y_sb[:, c, :].bitcast(F32),
                        in1=y_sb[:, c, :].bitcast(F32),
                    )
                    nc.tensor.matmul(
                        out=sy2_ps[:, :], lhsT=ones128[:, :],
                        rhs=ysq[:, :], start=(c == 0), stop=(c == 1),
                    )
                mu2row = row_pool.tile([1, ql], F32, tag="mu2row")
                nc.vector.tensor_mul(
                    out=mu2row[:, :], in0=murow[:, :], in1=murow[:, :],
                )
                varrow = row_pool.tile([1, ql], F32, tag="varrow")
                nc.vector.scalar_tensor_tensor(
                    out=varrow[:, :], in0=mu2row[:, :], scalar=-float(C),
                    in1=sy2_ps[:, :], op0=OP.mult, op1=OP.add,
                )
                lnv = row_pool.tile([1, ql], F32, tag="lnv")
                nc.scalar.activation(
                    out=lnv[:, :], in_=varrow[:, :], func=AF.Ln,
                    scale=1.0 / C, bias=epsb[:, :],
                )
                rstdrow = row_pool.tile([1, ql], F32R, tag="rstdrow")
                nc.scalar.activation(
                    out=rstdrow[:, :], in_=lnv[:, :], func=AF.Exp, scale=-0.5
                )
                mu_rep = scr_pool.tile([128, ql], F32, tag="murep")
                nc.gpsimd.partition_broadcast(mu_rep[:, :], murow[:, :])
                rs_ps = acc_ps.tile([128, ql], F32, tag="acc")
                nc.tensor.matmul(
                    out=rs_ps[:, :], lhsT=ones_col[:, :],
                    rhs=rstdrow[:, :], start=True, stop=True,
                )
                for c in range(2):
                    yn = scr_pool.tile([128, ql], F32, tag="scr")
                    nc.vector.tensor_sub(
                        out=yn[:, :],
                        in0=y_sb[:, c, :].bitcast(F32),
                        in1=mu_rep[:, :],
                    )
                    nc.vector.tensor_mul(
                        out=yn[:, :], in0=yn[:, :], in1=rs_ps[:, :]
                    )
                    osb = out_pool.tile([128, ql], F32)
                    nc.vector.tensor_scalar(
                        out=osb[:, :], in0=yn[:, :],
                        scalar1=pvec[:, ds(LNG + c, 1)],
                        scalar2=pvec[:, ds(LNB + c, 1)],
                        op0=OP.mult, op1=OP.add,
                    )
                    nc.sync.dma_start(
                        out=out_d[ds(c * 128, 128), qsl], in_=osb[:, :]
                    )

            # ---------------- preamble: projections woven with block-0
            # S pairs so ACT's exp stream starts ~10us in ----------------
            quarters[0] = alloc_quarters(0)
            k_unit(0, 0, split_j=True)
            q_proj(0)
            k_unit(0, 1)
            s_pair(0, 0)
            k_unit(1, 0)
            s_pair(0, 1)
            k_unit(1, 1)
            s_pair(0, 2)
            k_unit(2, 0)
            s_pair(0, 3)
            k_unit(2, 1)
            s_pair(0, 4)
            k_unit(3, 0)
            s_pair(0, 5)
            k_unit(3, 1)
            s_pair(0, 6)
            for up in range(4):
                v_unit(0, up)
            s_pair(0, 7)
            for r in range(1, NKR):
                for up in range(4):
                    v_unit(r, up)
            for qb4 in range(1, NQB):
                q_proj(qb4)
            dps = {0: denom(0)}

            # ---------------- steady state ----------------
            for b in range(NQB):
                nb = b + 1
                last = nb == NQB
                # 1/denom: single custom-DVE op (ACT stays exp-only)
                rcprow = row_pool.tile([1, QB], F32, tag="rcprow",
                                       name=f"rcprow{b}")
                nc.vector.reciprocal_approx_fast(
                    out=rcprow[:, :], in_=dps[b][:, :]
                )
                rcp_rep = scr_pool.tile([128, QB], F32, tag="rcprep",
                                        name=f"rcprep{b}")
                nc.gpsimd.partition_broadcast(rcp_rep[:, :], rcprow[:, :])
                if not last:
                    quarters[nb] = alloc_quarters(nb)
                    s_pair(nb, 0)
                ot = ot_pool.tile([128, 2, QB], F8, tag="ot", name=f"ot{b}")
                ops0 = acc_ps.tile([128, QB], F32, tag="acc")
                pv_part(b, 0, 0, 8, ops0)
                if not last:
                    s_pair(nb, 1)
                pv_part(b, 0, 8, 16, ops0)
                if last:
                    nc.scalar.activation(out=ot[:, 0, :], in_=ops0[:, :],
                                         func=AF.Copy)
                else:
                    nc.vector.tensor_copy(ot[:, 0, :], ops0[:, :])
                if not last:
                    s_pair(nb, 2)
                ops1 = acc_ps.tile([128, QB], F32, tag="acc")
                pv_part(b, 1, 0, 8, ops1)
                if not last:
                    s_pair(nb, 3)
                pv_part(b, 1, 8, 16, ops1)
                if last:
                    nc.scalar.activation(out=ot[:, 1, :], in_=ops1[:, :],
                                         func=AF.Copy)
                else:
                    nc.vector.tensor_copy(ot[:, 1, :], ops1[:, :])
                if last:
                    # tail halves: half 2's matmuls overlap half 1's
                    # serial DVE/ACT chain
                    for half in range(2):
                        y_h = outproj_y(b, ot, rcp_rep, qo=half * 256, ql=256)
                        stats_ln_last(b, y_h, qo=half * 256, ql=256)
                else:
                    s_pair(nb, 4)
                    y_b = outproj_y(b, ot, rcp_rep)
                    s_pair(nb, 5)
                    murow, mu_rep = stats_ln_a(b, y_b)
                    s_pair(nb, 6)
                    rs_rep = stats_ln_b(b, y_b, murow)
                    s_pair(nb, 7)
                    stats_ln_c(b, y_b, mu_rep, rs_rep)
                    dps[nb] = denom(nb)

    # Force Exp and Ln to resolve to the one table set containing both
    # (the default chooser alternates exp_and_others <-> natural_log_exp,
    # paying a ~1.3us table load per switch, ~17 loads per kernel).
    import bass_rust as _br
    from concourse.hw_specs import get_activation_tables as _gat

    def _patched_act_loads():
        has_act = any(
            isinstance(i, mybir.InstActivation)
            for blk in nc.main_func.blocks for i in blk.instructions
        )
        if not has_act:
            return
        tables = []
        for name, fns in _gat(nc.m.arch).items():
            if name != "natural_log_exp_and_others":
                fns = fns - {AF.Exp, AF.Ln}
            tables.append((name, fns))
        _br.insert_act_table_loads(nc, tables)

    nc.insert_act_table_loads = _patched_act_loads
    nc.compile()
    return nc


def get_nc():
    if "nc" not in _CACHE:
        _CACHE["nc"] = _build_nc()
    return _CACHE["nc"]


def make_in_maps(low, high, q_w, q_b, k_w, k_b, v_w, v_b, o_w, o_b, ln_g, ln_b):
    low_r = np.asarray(low, np.float32).reshape(B, C, N)
    high_r = np.asarray(high, np.float32).reshape(B, C, N)
    f32 = lambda x: np.ascontiguousarray(np.asarray(x, np.float32))
    f8 = lambda x: np.ascontiguousarray(
        np.asarray(x, np.float32).astype(ml_dtypes.float8_e4m3)
    )
    # v-bias is exactly equivalent to an out-proj bias shift because the
    # softmax rows sum to one: attn @ (V + 1 vb^T) @ o_w^T = attn @ V @ o_w^T
    # + (o_w @ v_b)^T, so fold it on the host.
    ob_eff = np.asarray(o_b, np.float32) + np.asarray(o_w, np.float32) @ np.asarray(v_b, np.float32)
    pv_cols = []
    for v in [np.asarray(q_b, np.float32) * QK_PRE,
              np.asarray(k_b, np.float32) * QK_PRE,
              ob_eff, ln_g, ln_b]:
        pv_cols.append(np.asarray(v, np.float32).reshape(2, 128).T)
    shared = {
        "wq": f8(np.asarray(q_w, np.float32).T * QK_PRE),
        "wk": f8(np.asarray(k_w, np.float32).T * QK_PRE),
        "wv": f8(np.asarray(v_w, np.float32).T),
        "wo": f8(np.asarray(o_w, np.float32).T),
        "pvec": f32(np.concatenate(pv_cols, axis=1)),
    }
    in_maps = []
    for i in range(8):
        bidx, h = i // 2, i % 2
        lo = low_r[bidx][:, h * NQ:(h + 1) * NQ]
        in_maps.append({
            "low": f32(lo),
            "lowf8": f8(lo),
            "high": f8(high_r[bidx]),
            **shared,
        })
    return in_maps


def assemble(results):
    out = np.empty((B, C, N), np.float32)
    for i in range(8):
        bidx, h = i // 2, i % 2
        out[bidx][:, h * NQ:(h + 1) * NQ] = results[i]["out"]
    return out.reshape(B, C, 64, 64)


def kernel(**inputs) -> np.ndarray:
    nc = get_nc()
    in_maps = make_in_maps(**inputs)
    res = run_bass_kernel_spmd(nc, in_maps, core_ids=list(range(8)))
    return assemble(res.results)


if __name__ == "__main__":
    pass


# revision 8
# speedup vs baseline: 1.0396x; 1.0110x over previous
"""ContentGuidedAttention Trainium2 kernel.

Full NxN single-head cross-attention + out-proj + residual + LayerNorm,
for B=4, C=256, H=W=64 (N=4096 tokens), distributed over 8 NeuronCores:
core i handles batch i//2, query-half i%2 (2048 queries, all 4096 keys).
No collectives: K/V are computed redundantly on the two cores sharing a
batch (~5% extra FLOPs).

Layout strategy (channel-major, zero transposes, fp8 DoubleRow on every
matmul the PE streams):
  - Q^T/K^T as [C, n] fp8e4 (q/k weights prescaled by 16, compensated in
    the softmax exp scale); V token-major [n, C] fp8e4 via DR matmuls,
    evacuated on GpSimd (DVE is the preamble bottleneck)
  - S^T = K Q^T fp8 DR; exp on ACT -> P^T fp8e4
  - softmax denominator: DR ones-vector matmuls accumulate into a [1, q]
    psum row; 1/denom via a single DVE reciprocal_approx_fast (keeps the
    ACT queue exp-only), then gpsimd partition_broadcast
  - PV: O^T[c, q] = sum_k V[k,c] P^T[k,q], fp8 DR; O^T evacuated to fp8
    so the out-proj is DR too; residual uses a separate f32 copy of low
  - scheduling: the steady-state loop WEAVES 2-si S-matmul pairs between
    ~1.5us chunks of PV/outproj/LN/denominator matmuls so the in-order
    PE queue never parks behind an S matmul whose psum buffer is still
    being drained by ACT (st_ps has only 2 bufs), and ACT always has exp
    work queued.  Same fine-grained interleave in the projection
    preamble.  ~35 dummy 128-col matmuls at t~3.5us warm the PE HAM
    clock-gate before real work; inputs ride 4 DMA queues in parallel.
  - tail: last block's out-proj + LN run as two 256-query halves so the
    serial ACT/DVE chain of one half overlaps the other's matmuls.
"""

import ml_dtypes
import numpy as np

import concourse.bass as bass
import concourse.mybir as mybir
import concourse.tile as tile
from concourse import bacc
from concourse.bass import ds, ts
from concourse.bass_utils import run_bass_kernel_spmd

F32 = mybir.dt.float32
F32R = mybir.dt.float32r
BF16 = mybir.dt.bfloat16
F8 = mybir.dt.float8e4
AF = mybir.ActivationFunctionType
OP = mybir.AluOpType
DR = mybir.MatmulPerfMode.DoubleRow

B = 4
C = 256
N = 4096          # tokens per batch
NQ = 2048         # queries per core
QB = 512          # query block
NQB = NQ // QB    # 4
NKC = N // 128    # 32 key chunks
NKR = 4           # key ranges (1024 keys each) for K^T / V tiles
QK_PRE = 16.0     # host-side prescale on q/k weights (fp8 range centering)
SCALE = (C // 8) ** -0.5
EXP_SCALE = SCALE / (QK_PRE * QK_PRE)
LN_EPS = 1e-5

_CACHE = {}


def _build_nc():
    nc = bacc.Bacc("TRN2", target_bir_lowering=False, debug=False)

    low_d = nc.declare_dram_parameter("low", [C, NQ], F32R, isOutput=False)
    lowf8_d = nc.declare_dram_parameter("lowf8", [C, NQ], F8, isOutput=False)
    high_d = nc.declare_dram_parameter("high", [C, N], F8, isOutput=False)
    # weights are passed pre-transposed: [c_in, c_out], fp8
    wq_d = nc.declare_dram_parameter("wq", [C, C], F8, isOutput=False)
    wk_d = nc.declare_dram_parameter("wk", [C, C], F8, isOutput=False)
    wv_d = nc.declare_dram_parameter("wv", [C, C], F8, isOutput=False)
    wo_d = nc.declare_dram_parameter("wo", [C, C], F8, isOutput=False)
    # qb, kb, ob, lng, lnb prepacked host-side as [128, 10]
    pvec_d = nc.declare_dram_parameter("pvec", [128, 10], F32, isOutput=False)
    out_d = nc.declare_dram_parameter("out", [C, NQ], F32, isOutput=True)

    with tile.TileContext(nc) as tc:
        with (
            tc.tile_pool(name="persist", bufs=1) as pp,
            tc.tile_pool(name="high", bufs=4) as high_pool,
            tc.tile_pool(name="pt", bufs=8) as pt_pool,
            tc.tile_pool(name="ot", bufs=3) as ot_pool,
            tc.tile_pool(name="scratch", bufs=3) as scr_pool,
            tc.tile_pool(name="rowscr", bufs=1) as row_pool,
            tc.tile_pool(name="outsb", bufs=4) as out_pool,
            tc.tile_pool(name="st_ps", bufs=2, space="PSUM") as st_ps,
            tc.tile_pool(name="acc_ps", bufs=3, space="PSUM") as acc_ps,
            tc.tile_pool(name="row_ps", bufs=1, space="PSUM") as row_ps,
        ):
            # ---------------- constants + PE warm-up ----------------
            stage = pp.tile([128, 128], F32)
            nc.vector.memset(stage[:, :], 1.0)
            ones128 = pp.tile([128, 1], F32R)    # partition-reduce lhsT
            nc.vector.tensor_copy(ones128[:, :], stage[:, 0:1])
            # tiny exp: pulls the ACT table load to ~3.6us (ACT idle)
            tinyrow = pp.tile([1, 1], F32)
            nc.scalar.activation(out=tinyrow[:, :], in_=stage[0:1, 0:1],
                                 func=AF.Exp)
            # ~35 dummy matmuls release the PE HAM clock-gate (~3.4us of
            # sustained activity) before the first real matmul at ~8us
            warm_ps = row_ps.tile([1, 128], F32, tag="row")
            for w in range(35):
                nc.tensor.matmul(
                    out=warm_ps[:, :], lhsT=ones128[:, :],
                    rhs=stage[:, :].bitcast(F32R),
                    start=True, stop=True, skip_group_check=True,
                )
            ones2f8 = pp.tile([128, 2, 16], F8)  # DoubleRow denom lhsT
            nc.vector.tensor_copy(ones2f8[:, :, 0], stage[:, 0:2])
            ones_col = pp.tile([1, 128], F32R)   # K=1 row-broadcast lhsT
            nc.vector.tensor_copy(ones_col[:, :], stage[0:1, :])
            epsb = pp.tile([1, 1], F32)          # LN epsilon bias
            nc.vector.memset(epsb[:, :], LN_EPS)

            # ---------------- input DMAs on 4 parallel queues --------
            wk_sb = pp.tile([128, 2, C], F8)
            wv_sb = pp.tile([128, 2, C], F8)
            wq_sb = pp.tile([128, 2, C], F8)
            wo_sb = pp.tile([128, 2, C], F8)
            pvec = pp.tile([128, 10], F32)
            lowf8_sb = pp.tile([128, 2, NQ], F8)
            low_sb = pp.tile([128, 2, NQ], F32R)
            for j in range(2):
                nc.scalar.dma_start(out=wk_sb[:, j, :], in_=wk_d[ds(j * 128, 128), :])
            nc.gpsimd.dma_start(out=pvec[:, :], in_=pvec_d[:, :])
            for j in range(2):
                nc.gpsimd.dma_start(out=wq_sb[:, j, :], in_=wq_d[ds(j * 128, 128), :])
            for j in range(2):
                nc.gpsimd.dma_start(out=wv_sb[:, j, :], in_=wv_d[ds(j * 128, 128), :])
            for j in range(2):
                nc.gpsimd.dma_start(out=wo_sb[:, j, :], in_=wo_d[ds(j * 128, 128), :])
            hi_tiles = [
                high_pool.tile([128, 2, 1024], F8, name=f"hi{r}")
                for r in range(NKR)
            ]
            # range 0 rides first, split in h-halves so the very first
            # K-projection matmul waits on a [128,512] transfer only
            for h in range(2):
                for j in range(2):
                    nc.sync.dma_start(
                        out=hi_tiles[0][:, j, ds(h * 512, 512)],
                        in_=high_d[ds(j * 128, 128), ds(h * 512, 512)],
                    )
            for j in range(2):
                nc.sync.dma_start(out=lowf8_sb[:, j, :], in_=lowf8_d[ds(j * 128, 128), :])
            for r in range(1, NKR):
                for j in range(2):
                    nc.sync.dma_start(
                        out=hi_tiles[r][:, j, :],
                        in_=high_d[ds(j * 128, 128), ds(r * 1024, 1024)],
                    )
            for j in range(2):
                nc.sync.dma_start(out=low_sb[:, j, :], in_=low_d[ds(j * 128, 128), :])

            QBIAS, KBIAS, OBIAS, LNG, LNB = 0, 2, 4, 6, 8

            kt_sb = [
                pp.tile([128, 2, 1024], F8, name=f"kt{r}", tag=f"kt{r}")
                for r in range(NKR)
            ]
            v_sb = [
                pp.tile([128, 8, C], F8, name=f"v{r}", tag=f"v{r}")
                for r in range(NKR)
            ]
            qt_all = pp.tile([128, 2, NQ], F8)

            # ---------------- work units ----------------
            def k_unit(r, h, split_j=False):
                # K^T: out [cout, k] = sum_cin wk[cin, cout] high[cin, k]
                for c in range(2):
                    kps = st_ps.tile([128, 512], F32, tag="st")
                    if split_j:
                        # first matmuls only need the first DMA chunks
                        for j in range(2):
                            nc.tensor.matmul(
                                out=kps[:, :],
                                lhsT=wk_sb[:, j, ds(c * 128, 128)],
                                rhs=hi_tiles[r][:, j, ds(h * 512, 512)],
                                start=(j == 0), stop=(j == 1),
                            )
                    else:
                        nc.tensor.matmul(
                            out=kps[:, :],
                            lhsT=wk_sb[:, :, ds(c * 128, 128)],
                            rhs=hi_tiles[r][:, :, ds(h * 512, 512)],
                            start=True, stop=True,
                            perf_mode=DR,
                        )
                    # K bias dropped: a k-independent logit shift per query,
                    # exactly cancelled by softmax
                    nc.vector.tensor_copy(
                        kt_sb[r][:, c, ds(h * 512, 512)], kps[:, :]
                    )

            def v_unit(r, up):
                # V: out [k, cout] = sum_cin high[cin, k] wv[cin, cout]
                # DR over the cin halves; last range evacuates on ACT to
                # balance the preamble DVE load
                vps = st_ps.tile([128, 2, C], F32, tag="st")
                for i in range(2):
                    u = up * 2 + i
                    nc.tensor.matmul(
                        out=vps[:, i, :],
                        lhsT=hi_tiles[r][:, :, ds(u * 128, 128)],
                        rhs=wv_sb[:, :, :],
                        start=True, stop=True,
                        perf_mode=DR,
                    )
                if r == NKR - 1:
                    nc.scalar.activation(
                        out=v_sb[r][:, ds(up * 2, 2), :], in_=vps[:, :, :],
                        func=AF.Copy,
                    )
                else:
                    nc.vector.tensor_copy(
                        v_sb[r][:, ds(up * 2, 2), :], vps[:, :, :]
                    )

            def q_proj(qb4):
                for c in range(2):
                    qps = st_ps.tile([128, QB], F32, tag="st")
                    nc.tensor.matmul(
                        out=qps[:, :],
                        lhsT=wq_sb[:, :, ds(c * 128, 128)],
                        rhs=lowf8_sb[:, :, ds(qb4 * QB, QB)],
                        start=True, stop=True,
                        perf_mode=DR,
                    )
                    nc.vector.tensor_scalar_add(
                        out=qt_all[:, c, ds(qb4 * QB, QB)], in0=qps[:, :],
                        scalar1=pvec[:, ds(QBIAS + c, 1)],
                    )

            def alloc_quarters(b):
                return [
                    pt_pool.tile([128, 8, QB], F8, tag="ptq", name=f"ptq{g}")
                    for g in range(4)
                ]

            quarters = {}

            def s_pair(b, p):
                # 2 si = 4 key chunks: 4 S matmuls + 2 exps; sized so the
                # two st_ps bufs never park the in-order PE queue
                qsl = ds(b * QB, QB)
                for si in (2 * p, 2 * p + 1):
                    sps = st_ps.tile([128, 2, QB], F32, tag="st")
                    for u in range(2):
                        kc = si * 2 + u
                        nc.tensor.matmul(
                            out=sps[:, u, :],
                            lhsT=kt_sb[kc // 8][:, :, ds((kc % 8) * 128, 128)],
                            rhs=qt_all[:, :, qsl],
                            start=True, stop=True,
                            perf_mode=DR,
                        )
                    nc.scalar.activation(
                        out=quarters[b][si // 4][:, ds((si % 4) * 2, 2), :],
                        in_=sps[:, :, :],
                        func=AF.Exp,
                        scale=EXP_SCALE,
                    )

            def denom_part(b, t0, t1, dps=None):
                # split accumulation: t12-15 can be emitted after other PE
                # work so the last exps of block b have time to land
                if dps is None:
                    dps = row_ps.tile([1, QB], F32, tag="row")
                for t in range(t0, t1):
                    nc.tensor.matmul(
                        out=dps[:, :],
                        lhsT=ones2f8[:, :, 0:1],
                        rhs=quarters[b][t // 4][:, ds((t % 4) * 2, 2), :],
                        start=(t == 0), stop=(t == t1 - 1),
                        perf_mode=DR,
                        skip_group_check=True,
                    )
                return dps

            def pv_part(b, c, t0, t1, ops):
                for t in range(t0, t1):
                    nc.tensor.matmul(
                        out=ops[:, :],
                        lhsT=v_sb[t // 4][:, ds((t % 4) * 2, 2),
                                         ds(c * 128, 128)],
                        rhs=quarters[b][t // 4][:, ds((t % 4) * 2, 2), :],
                        start=(t == 0), stop=(t == t1 - 1),
                        perf_mode=DR,
                        skip_group_check=True,
                    )

            def outproj_y(b, ot, rcp_rep, qo=0, ql=QB):
                qsl = ds(b * QB + qo, ql)
                y_sb = ot_pool.tile([128, 2, ql], F32R, tag="y",
                                    name=f"y{b}_{qo}")
                for c in range(2):
                    pps = acc_ps.tile([128, ql], F32, tag="acc")
                    nc.tensor.matmul(
                        out=pps[:, :],
                        lhsT=wo_sb[:, :, ds(c * 128, 128)],
                        rhs=ot[:, :, ds(qo, ql)],
                        start=True, stop=True,
                        perf_mode=DR,
                    )
                    ysc = scr_pool.tile([128, ql], F32, tag="scr")
                    nc.vector.tensor_mul(
                        out=ysc[:, :], in0=pps[:, :], in1=rcp_rep[:, ds(qo, ql)]
                    )
                    nc.vector.scalar_tensor_tensor(
                        out=y_sb[:, c, :],
                        in0=ysc[:, :],
                        scalar=pvec[:, ds(OBIAS + c, 1)],
                        in1=low_sb[:, c, qsl].bitcast(F32),
                        op0=OP.add, op1=OP.add,
                    )
                return y_sb

            def stats_ln_a(b, y_sb):
                sy_ps = row_ps.tile([1, QB], F32, tag="row")
                for c in range(2):
                    nc.tensor.matmul(
                        out=sy_ps[:, :], lhsT=ones128[:, :],
                        rhs=y_sb[:, c, :], start=(c == 0), stop=(c == 1),
                    )
                murow = row_pool.tile([1, QB], F32, tag="murow")
                nc.vector.tensor_scalar_mul(
                    out=murow[:, :], in0=sy_ps[:, :], scalar1=1.0 / C
                )
                mu_rep = scr_pool.tile([128, QB], F32, tag="murep")
                nc.gpsimd.partition_broadcast(mu_rep[:, :], murow[:, :])
                return murow, mu_rep

            def stats_ln_b(b, y_sb, murow):
                sy2_ps = row_ps.tile([1, QB], F32, tag="row")
                for c in range(2):
                    ysq = scr_pool.tile([128, QB], F32R, tag="ysq")
                    nc.vector.tensor_mul(
                        out=ysq[:, :],
                        in0=y_sb[:, c, :].bitcast(F32),
                        in1=y_sb[:, c, :].bitcast(F32),
                    )
                    nc.tensor.matmul(
                        out=sy2_ps[:, :], lhsT=ones128[:, :],
                        rhs=ysq[:, :], start=(c == 0), stop=(c == 1),
                    )
                # C*var = sy2 - C*mu^2 ; rstd = exp(-0.5 ln((C var)/C + eps))
                mu2row = row_pool.tile([1, QB], F32, tag="mu2row")
                nc.vector.tensor_mul(
                    out=mu2row[:, :], in0=murow[:, :], in1=murow[:, :],
                )
                varrow = row_pool.tile([1, QB], F32, tag="varrow")
                nc.vector.scalar_tensor_tensor(
                    out=varrow[:, :], in0=mu2row[:, :], scalar=-float(C),
                    in1=sy2_ps[:, :], op0=OP.mult, op1=OP.add,
                )
                lnv = row_pool.tile([1, QB], F32, tag="lnv")
                nc.scalar.activation(
                    out=lnv[:, :], in_=varrow[:, :], func=AF.Ln,
                    scale=1.0 / C, bias=epsb[:, :],
                )
                rstdrow = row_pool.tile([1, QB], F32, tag="rstdrow")
                nc.scalar.activation(
                    out=rstdrow[:, :], in_=lnv[:, :], func=AF.Exp, scale=-0.5
                )
                rs_rep = scr_pool.tile([128, QB], F32, tag="rsrep")
                nc.gpsimd.partition_broadcast(rs_rep[:, :], rstdrow[:, :])
                return rs_rep

            def stats_ln_c(b, y_sb, mu_rep, rs_rep):
                qsl = ds(b * QB, QB)
                for c in range(2):
                    yn = scr_pool.tile([128, QB], F32, tag="scr")
                    nc.vector.tensor_sub(
                        out=yn[:, :],
                        in0=y_sb[:, c, :].bitcast(F32),
                        in1=mu_rep[:, :],
                    )
                    nc.vector.tensor_mul(
                        out=yn[:, :], in0=yn[:, :], in1=rs_rep[:, :]
                    )
                    osb = out_pool.tile([128, QB], F32)
                    nc.vector.tensor_scalar(
                        out=osb[:, :], in0=yn[:, :],
                        scalar1=pvec[:, ds(LNG + c, 1)],
                        scalar2=pvec[:, ds(LNB + c, 1)],
                        op0=OP.mult, op1=OP.add,
                    )
                    nc.sync.dma_start(
                        out=out_d[ds(c * 128, 128), qsl], in_=osb[:, :]
                    )

            def stats_ln_last(b, y_sb, qo, ql):
                # span-critical tail: murow on ACT, rstd broadcast via a
                # K=1 PE matmul into psum (~0.3us vs ~1us gpsimd)
                qsl = ds(b * QB + qo, ql)
                sy_ps = row_ps.tile([1, ql], F32, tag="row")
                for c in range(2):
                    nc.tensor.matmul(
                        out=sy_ps[:, :], lhsT=ones128[:, :],
                        rhs=y_sb[:, c, :], start=(c == 0), stop=(c == 1),
                    )
                murow = row_pool.tile([1, ql], F32, tag="murow")
                nc.scalar.activation(
                    out=murow[:, :], in_=sy_ps[:, :], func=AF.Copy,
                    scale=1.0 / C,
                )
                sy2_ps = row_ps.tile([1, ql], F32, tag="row")
                for c in range(2):
                    ysq = scr_pool.tile([128, ql], F32R, tag="ysq")
                    nc.vector.tensor_mul(
                        out=ysq[:, :],
                        in0=y_sb[:, c, :].bitcast(F32),
                        in1=y_sb[:, c, :].bitcast(F32),
                    )
                    nc.tensor.matmul(
                        out=sy2_ps[:, :], lhsT=ones128[:, :],
                        rhs=ysq[:, :], start=(c == 0), stop=(c == 1),
                    )
                mu2row = row_pool.tile([1, ql], F32, tag="mu2row")
                nc.vector.tensor_mul(
                    out=mu2row[:, :], in0=murow[:, :], in1=murow[:, :],
                )
                varrow = row_pool.tile([1, ql], F32, tag="varrow")
                nc.vector.scalar_tensor_tensor(
                    out=varrow[:, :], in0=mu2row[:, :], scalar=-float(C),
                    in1=sy2_ps[:, :], op0=OP.mult, op1=OP.add,
                )
                lnv = row_pool.tile([1, ql], F32, tag="lnv")
                nc.scalar.activation(
                    out=lnv[:, :], in_=varrow[:, :], func=AF.Ln,
                    scale=1.0 / C, bias=epsb[:, :],
                )
                rstdrow = row_pool.tile([1, ql], F32R, tag="rstdrow")
                nc.scalar.activation(
                    out=rstdrow[:, :], in_=lnv[:, :], func=AF.Exp, scale=-0.5
                )
                mu_rep = scr_pool.tile([128, ql], F32, tag="murep")
                nc.gpsimd.partition_broadcast(mu_rep[:, :], murow[:, :])
                rs_ps = acc_ps.tile([128, ql], F32, tag="acc")
                nc.tensor.matmul(
                    out=rs_ps[:, :], lhsT=ones_col[:, :],
                    rhs=rstdrow[:, :], start=True, stop=True,
                )
                for c in range(2):
                    yn = scr_pool.tile([128, ql], F32, tag="scr")
                    nc.vector.tensor_sub(
                        out=yn[:, :],
                        in0=y_sb[:, c, :].bitcast(F32),
                        in1=mu_rep[:, :],
                    )
                    nc.vector.tensor_mul(
                        out=yn[:, :], in0=yn[:, :], in1=rs_ps[:, :]
                    )
                    osb = out_pool.tile([128, ql], F32)
                    nc.vector.tensor_scalar(
                        out=osb[:, :], in0=yn[:, :],
                        scalar1=pvec[:, ds(LNG + c, 1)],
                        scalar2=pvec[:, ds(LNB + c, 1)],
                        op0=OP.mult, op1=OP.add,
                    )
                    nc.sync.dma_start(
                        out=out_d[ds(c * 128, 128), qsl], in_=osb[:, :]
                    )

            # ---------------- preamble: projections woven with block-0
            # S pairs so ACT's exp stream starts ~10us in ----------------
            quarters[0] = alloc_quarters(0)
            k_unit(0, 0, split_j=True)
            q_proj(0)
            k_unit(0, 1)
            s_pair(0, 0)
            k_unit(1, 0)
            s_pair(0, 1)
            k_unit(1, 1)
            s_pair(0, 2)
            k_unit(2, 0)
            s_pair(0, 3)
            k_unit(2, 1)
            s_pair(0, 4)
            k_unit(3, 0)
            s_pair(0, 5)
            k_unit(3, 1)
            s_pair(0, 6)
            for up in range(4):
                v_unit(0, up)
            s_pair(0, 7)
            for r in range(1, NKR):
                for up in range(4):
                    v_unit(r, up)
            for qb4 in range(1, NQB):
                q_proj(qb4)
            dps = {0: denom_part(0, 0, 16)}

            # ---------------- steady state (b = 0..2) ----------------
            # block 3's PV t0-11 is pre-woven into iteration b=2 so the
            # final iteration is just 8 matmuls + the LN tail chain
            pv3 = {}
            for b in range(NQB - 1):
                nb = b + 1
                # 1/denom: single custom-DVE op (ACT stays exp-only)
                rcprow = row_pool.tile([1, QB], F32, tag="rcprow",
                                       name=f"rcprow{b}")
                nc.vector.reciprocal_approx_fast(
                    out=rcprow[:, :], in_=dps[b][:, :]
                )
                rcp_rep = scr_pool.tile([128, QB], F32, tag="rcprep",
                                        name=f"rcprep{b}")
                nc.gpsimd.partition_broadcast(rcp_rep[:, :], rcprow[:, :])
                quarters[nb] = alloc_quarters(nb)
                s_pair(nb, 0)
                ot = ot_pool.tile([128, 2, QB], F8, tag="ot", name=f"ot{b}")
                ops0 = acc_ps.tile([128, QB], F32, tag="acc")
                pv_part(b, 0, 0, 8, ops0)
                s_pair(nb, 1)
                pv_part(b, 0, 8, 16, ops0)
                nc.vector.tensor_copy(ot[:, 0, :], ops0[:, :])
                s_pair(nb, 2)
                ops1 = acc_ps.tile([128, QB], F32, tag="acc")
                pv_part(b, 1, 0, 8, ops1)
                s_pair(nb, 3)
                pv_part(b, 1, 8, 16, ops1)
                nc.vector.tensor_copy(ot[:, 1, :], ops1[:, :])
                s_pair(nb, 4)
                y_b = outproj_y(b, ot, rcp_rep)
                s_pair(nb, 5)
                murow, mu_rep = stats_ln_a(b, y_b)
                s_pair(nb, 6)
                if b == 2:
                    # pre-run block 3's PV while its exps are landing
                    pv3["ops0"] = acc_ps.tile([128, QB], F32, tag="acc",
                                              name="pv3c0")
                    pv_part(3, 0, 0, 8, pv3["ops0"])
                rs_rep = stats_ln_b(b, y_b, murow)
                s_pair(nb, 7)
                if b == 2:
                    pv_part(3, 0, 8, 12, pv3["ops0"])
                    pv3["ops1"] = acc_ps.tile([128, QB], F32, tag="acc",
                                              name="pv3c1")
                    pv_part(3, 1, 0, 8, pv3["ops1"])
                dps[nb] = denom_part(nb, 0, 12)
                stats_ln_c(b, y_b, mu_rep, rs_rep)
                if b == 2:
                    pv_part(3, 1, 8, 12, pv3["ops1"])
                denom_part(nb, 12, 16, dps[nb])

            # ---------------- tail: block 3 ----------------
            b = 3
            # recip on the now-idle ACT; dps[3] is complete
            lnd = row_pool.tile([1, QB], F32, tag="lnd")
            nc.scalar.activation(out=lnd[:, :], in_=dps[3][:, :], func=AF.Ln)
            rcprow3 = row_pool.tile([1, QB], F32, tag="rcprow3")
            nc.scalar.activation(out=rcprow3[:, :], in_=lnd[:, :],
                                 func=AF.Exp, scale=-1.0)
            rcp_rep3 = scr_pool.tile([128, QB], F32, tag="rcprep",
                                     name="rcprep3")
            nc.gpsimd.partition_broadcast(rcp_rep3[:, :], rcprow3[:, :])
            ot = ot_pool.tile([128, 2, QB], F8, tag="ot", name="ot3")
            pv_part(3, 0, 12, 16, pv3["ops0"])
            nc.scalar.activation(out=ot[:, 0, :], in_=pv3["ops0"][:, :],
                                 func=AF.Copy)
            pv_part(3, 1, 12, 16, pv3["ops1"])
            nc.scalar.activation(out=ot[:, 1, :], in_=pv3["ops1"][:, :],
                                 func=AF.Copy)
            # tail halves: half 2's matmuls overlap half 1's serial chain
            for half in range(2):
                y_h = outproj_y(b, ot, rcp_rep3, qo=half * 256, ql=256)
                stats_ln_last(b, y_h, qo=half * 256, ql=256)

    # Force Exp and Ln to resolve to the one table set containing both
    # (the default chooser alternates exp_and_others <-> natural_log_exp,
    # paying a ~1.3us table load per switch, ~17 loads per kernel).
    import bass_rust as _br
    from concourse.hw_specs import get_activation_tables as _gat

    def _patched_act_loads():
        has_act = any(
            isinstance(i, mybir.InstActivation)
            for blk in nc.main_func.blocks for i in blk.instructions
        )
        if not has_act:
            return
        tables = []
        for name, fns in _gat(nc.m.arch).items():
            if name != "natural_log_exp_and_others":
                fns = fns - {AF.Exp, AF.Ln}
            tables.append((name, fns))
        _br.insert_act_table_loads(nc, tables)

    nc.insert_act_table_loads = _patched_act_loads
    nc.compile()
    return nc


def get_nc():
    if "nc" not in _CACHE:
        _CACHE["nc"] = _build_nc()
    return _CACHE["nc"]


def make_in_maps(low, high, q_w, q_b, k_w, k_b, v_w, v_b, o_w, o_b, ln_g, ln_b):
    low_r = np.asarray(low, np.float32).reshape(B, C, N)
    high_r = np.asarray(high, np.float32).reshape(B, C, N)
    f32 = lambda x: np.ascontiguousarray(np.asarray(x, np.float32))
    f8 = lambda x: np.ascontiguousarray(
        np.asarray(x, np.float32).astype(ml_dtypes.float8_e4m3)
    )
    # v-bias is exactly equivalent to an out-proj bias shift because the
    # softmax rows sum to one: attn @ (V + 1 vb^T) @ o_w^T = attn @ V @ o_w^T
    # + (o_w @ v_b)^T, so fold it on the host.
    ob_eff = np.asarray(o_b, np.float32) + np.asarray(o_w, np.float32) @ np.asarray(v_b, np.float32)
    pv_cols = []
    for v in [np.asarray(q_b, np.float32) * QK_PRE,
              np.asarray(k_b, np.float32) * QK_PRE,
              ob_eff, ln_g, ln_b]:
        pv_cols.append(np.asarray(v, np.float32).reshape(2, 128).T)
    shared = {
        "wq": f8(np.asarray(q_w, np.float32).T * QK_PRE),
        "wk": f8(np.asarray(k_w, np.float32).T * QK_PRE),
        "wv": f8(np.asarray(v_w, np.float32).T),
        "wo": f8(np.asarray(o_w, np.float32).T),
        "pvec": f32(np.concatenate(pv_cols, axis=1)),
    }
    in_maps = []
    for i in range(8):
        bidx, h = i // 2, i % 2
        lo = low_r[bidx][:, h * NQ:(h + 1) * NQ]
        in_maps.append({
            "low": f32(lo),
            "lowf8": f8(lo),
            "high": f8(high_r[bidx]),
            **shared,
        })
    return in_maps


def assemble(results):
    out = np.empty((B, C, N), np.float32)
    for i in range(8):
        bidx, h = i // 2, i % 2
        out[bidx][:, h * NQ:(h + 1) * NQ] = results[i]["out"]
    return out.reshape(B, C, 64, 64)


def kernel(**inputs) -> np.ndarray:
    nc = get_nc()
    in_maps = make_in_maps(**inputs)
    res = run_bass_kernel_spmd(nc, in_maps, core_ids=list(range(8)))
    return assemble(res.results)


if __name__ == "__main__":
    pass
